# Initial kernel scaffold
#
"""Self-contained Bass/Tile SPMD kernel for nn_AIA_1_56049323213170 (8 NeuronCores).

Pipeline (B=2, C=256, H=W=128), all heavy math on-device in bf16/f32-psum:
  M1 = Xc @ Xr (CxC, contraction sharded 8-way + AllReduce)
  a  = rowsoftmax(M1)  (redundant per core, unnormalized + row-recip trick)
  s  = a @ Xc          (sharded: rows by batch, cols by quarter -> (128, 8192))
  rowsoftmax(s) needs only a global row-sum (|s|<=5.5 -> shift-free exp):
       AllReduce of per-core row sums within each batch group of 4 cores
  cia = x + softmax_W(rowsoftmax(s))   (W-softmax local, shift-free)
  AllGather cia within batch group -> full cia[b] per core
  y   = BN(conv3x3s2(x)+b), yc = BN(conv3x3s2(cia)+b)   (64 out-ch per core)
  branch = relu(y); x4_3 = sigmoid(lrelu(y,.2)); x1_2 = relu(yc)
  att1 = rowsoftmax(x1_2 @ branch^T); att2 = rowsoftmax(branch @ x4_3^T)
  x3_3 = rowsoftmax(x1_2 @ att2^T)  (shift-free)
  out  = bilinear_up2(relu(x3_3 + att1 + branch)) via R @ S @ R^T

Core p: b = p//4 (batch), q = p%4 (quarter; parity=q//2, h-half=q%2).
Each core returns out[b, 64q:64q+64] as bf16; host assembles f32.
"""
import numpy as np
import ml_dtypes

N_CORES = 8
B, C, H, W = 2, 256, 128, 128
HO = WO = 64
EPS = 1e-5

F32 = None  # set after imports
BF16 = None


def _resize_mat(n_out, n_in):
    R = np.zeros((n_out, n_in), np.float32)
    scale = n_in / n_out
    for i in range(n_out):
        src = (i + 0.5) * scale - 0.5
        i0 = int(np.floor(src))
        frac = src - i0
        lo = min(max(i0, 0), n_in - 1)
        hi = min(max(i0 + 1, 0), n_in - 1)
        R[i, lo] += 1.0 - frac
        R[i, hi] += frac
    return R


def _ap_of(t):
    """Normalize a tensor handle / tile / AP to a bass.AP."""
    import concourse.bass as bass
    if isinstance(t, bass.AP):
        return t
    return t.ap()


def _rap(t, offset, dims):
    """Raw AP over tensor handle t with [step,count] dims (elements)."""
    import concourse.bass as bass
    base = _ap_of(t)
    return bass.AP(tensor=base.tensor, offset=base.offset + offset,
                   ap=[list(d) for d in dims])


def _bcast_last(t, n):
    """AP of 2-D tile t broadcast with a 0-step innermost dim of size n."""
    import concourse.bass as bass
    base = _ap_of(t)
    return bass.AP(tensor=base.tensor, offset=base.offset,
                   ap=[list(base.ap[0]), list(base.ap[1]), [0, n]])


def build_nc():
    from contextlib import ExitStack
    import concourse.bass as bass
    import concourse.mybir as mybir
    import concourse.tile as tile
    from concourse import bacc
    from concourse.masks import make_identity

    f32 = mybir.dt.float32
    bf16 = mybir.dt.bfloat16
    AF = mybir.ActivationFunctionType
    AX = mybir.AxisListType
    ALU = mybir.AluOpType

    nc = bacc.Bacc("TRN2", target_bir_lowering=False, debug=False,
                   num_devices=N_CORES)

    # ---- I/O ----
    xb = nc.dram_tensor("xb", [C, H, W], bf16, kind="ExternalInput")
    xcT = nc.dram_tensor("xcT", [4096, 256], bf16, kind="ExternalInput")
    xr = nc.dram_tensor("xr", [4096, 256], bf16, kind="ExternalInput")
    xcm2 = nc.dram_tensor("xcm2", [128, 2, 8192], bf16, kind="ExternalInput")
    xblk = nc.dram_tensor("xblk", [128, 8192], bf16, kind="ExternalInput")
    sel = nc.dram_tensor("sel", [128, 2, 128], bf16, kind="ExternalInput")
    wt = nc.dram_tensor("wt", [128, 2, 9, 64], bf16, kind="ExternalInput")
    scale_d = nc.dram_tensor("scale", [64, 1], f32, kind="ExternalInput")
    shift_d = nc.dram_tensor("shift", [64, 1], f32, kind="ExternalInput")
    rt_d = nc.dram_tensor("rt", [64, 128], bf16, kind="ExternalInput")
    out_sh = nc.dram_tensor("out_sh", [64, H, W], bf16, kind="ExternalOutput")

    # ---- collective scratch ----
    cc1_in = nc.dram_tensor("cc1_in", [256, 256], f32)
    cc1_out = nc.dram_tensor("cc1_out", [256, 256], f32, addr_space="Shared")
    cc2_in = nc.dram_tensor("cc2_in", [128, 1], f32)
    cc2_out = nc.dram_tensor("cc2_out", [128, 1], f32)
    cc3_in = nc.dram_tensor("cc3_in", [128, 8192], bf16)
    cc3_out = nc.dram_tensor("cc3_out", [4, 128, 8192], bf16)
    G8 = [list(range(8))]
    G4 = [[0, 1, 2, 3], [4, 5, 6, 7]]

    with tile.TileContext(nc) as tc, ExitStack() as ctx:
        consts = ctx.enter_context(tc.tile_pool(name="consts", bufs=1))
        awork = ctx.enter_context(tc.tile_pool(name="awork", bufs=1))
        big = ctx.enter_context(tc.tile_pool(name="big", bufs=3))
        convout = ctx.enter_context(tc.tile_pool(name="convout", bufs=2))
        psA = ctx.enter_context(tc.tile_pool(name="psA", bufs=4, space="PSUM"))
        psB = ctx.enter_context(tc.tile_pool(name="psB", bufs=4, space="PSUM"))

        def psa():
            return psA.tile([128, 512], f32, tag="psA", name="psA_t")

        def psb():
            return psB.tile([128, 512], f32, tag="psB", name="psB_t")

        def psbt():
            return psB.tile([128, 512], bf16, tag="psB", name="psBt_t")

        # ================= constants =================
        id64 = consts.tile([64, 64], bf16, tag="id64")
        make_identity(nc, id64)
        id128 = consts.tile([128, 128], bf16, tag="id128")
        make_identity(nc, id128)
        rt_sb = consts.tile([64, 128], bf16, tag="rt")
        nc.sync.dma_start(out=rt_sb, in_=rt_d.ap())
        sel_sb = consts.tile([128, 2, 128], bf16, tag="sel")
        nc.sync.dma_start(out=sel_sb, in_=sel.ap())
        wt_sb = consts.tile([128, 2, 9, 64], bf16, tag="wt")
        nc.sync.dma_start(out=wt_sb, in_=wt.ap())
        scale_sb = consts.tile([64, 1], f32, tag="scale")
        nc.sync.dma_start(out=scale_sb, in_=scale_d.ap())
        shift_sb = consts.tile([64, 1], f32, tag="shift")
        nc.sync.dma_start(out=shift_sb, in_=shift_d.ap())
        xcm2_sb = consts.tile([128, 2, 8192], bf16, tag="xcm2")
        nc.sync.dma_start(out=xcm2_sb, in_=xcm2.ap())
        xblk_sb = consts.tile([128, 8192], bf16, tag="xblk")
        nc.sync.dma_start(out=xblk_sb, in_=xblk.ap())

        # ================= Phase A: M1 + softmax(a) =================
        xcT_sb = big.tile([128, 32, 256], bf16, tag="big8k", name="xcT_sb")
        nc.sync.dma_start(out=xcT_sb,
                          in_=xcT.ap().rearrange("(t p) c -> p t c", p=128))
        xr_sb = big.tile([128, 32, 256], bf16, tag="big8k", name="xr_sb")
        nc.sync.dma_start(out=xr_sb,
                          in_=xr.ap().rearrange("(t p) c -> p t c", p=128))

        m1ps = [psa(), psa()]
        for mc in range(2):
            for t in range(32):
                nc.tensor.matmul(
                    m1ps[mc][:, 0:256],
                    lhsT=xcT_sb[:, t, 128 * mc:128 * mc + 128],
                    rhs=xr_sb[:, t, :],
                    start=(t == 0), stop=(t == 31))
        for mc in range(2):
            m1e = awork.tile([128, 256], f32, tag="m1e", bufs=2)
            nc.vector.tensor_copy(out=m1e, in_=m1ps[mc][:, 0:256])
            nc.sync.dma_start(out=cc1_in.ap()[128 * mc:128 * mc + 128, :],
                              in_=m1e)
        nc.gpsimd.collective_compute(
            "AllReduce", ALU.add, replica_groups=G8,
            ins=[cc1_in.ap()], outs=[cc1_out.ap()])

        a_bf = []
        rinv = []
        for ch in range(2):
            a_raw = awork.tile([128, 256], f32, tag="a_raw")
            nc.sync.dma_start(out=a_raw,
                              in_=cc1_out.ap()[128 * ch:128 * ch + 128, :])
            negmax = awork.tile([128, 1], f32, tag="negmax")
            nc.vector.tensor_reduce(out=negmax, in_=a_raw, axis=AX.X,
                                    op=ALU.max, negate=True)
            a_e = awork.tile([128, 256], bf16, tag="a_e", bufs=2)
            asum = awork.tile([128, 1], f32, tag="asum")
            nc.scalar.activation(out=a_e, in_=a_raw, func=AF.Exp,
                                 bias=negmax, scale=1.0, accum_out=asum)
            ri = awork.tile([128, 1], f32, tag="ri", bufs=2)
            nc.vector.reciprocal(out=ri, in_=asum)
            a_bf.append(a_e)
            rinv.append(ri)

        # aT_own[c2,m] = sum_c1 a[c1,c2]*sel[c1,m];  rinv_own = sel^T @ rinv
        aTo = []
        for c2c in range(2):
            ps = psa()
            for c1c in range(2):
                nc.tensor.matmul(ps[:, 0:128],
                                 lhsT=a_bf[c1c][:, 128 * c2c:128 * c2c + 128],
                                 rhs=sel_sb[:, c1c, :],
                                 start=(c1c == 0), stop=(c1c == 1))
            t_ = awork.tile([128, 128], bf16, tag="aTo", bufs=2)
            nc.vector.tensor_copy(out=t_, in_=ps[:, 0:128])
            aTo.append(t_)
        sel_f = awork.tile([128, 2, 128], f32, tag="sel_f")
        nc.vector.tensor_copy(out=sel_f, in_=sel_sb)
        ps = psa()
        for c1c in range(2):
            nc.tensor.matmul(ps[:, 0:1], lhsT=sel_f[:, c1c, :],
                             rhs=rinv[c1c], start=(c1c == 0), stop=(c1c == 1))
        rinv_own = awork.tile([128, 1], f32, tag="rinv_own")
        nc.vector.tensor_copy(out=rinv_own, in_=ps[:, 0:1])

        # ================= Phase B: M2 -> E = exp(s) =================
        E_sb = big.tile([128, 8192], bf16, tag="big8k")
        partials = awork.tile([128, 16], f32, tag="partials")
        for nci in range(16):
            ps = psa()
            for bp in range(2):
                nc.tensor.matmul(ps,
                                 lhsT=aTo[bp],
                                 rhs=xcm2_sb[:, bp, 512 * nci:512 * nci + 512],
                                 start=(bp == 0), stop=(bp == 1))
            nc.scalar.activation(out=E_sb[:, 512 * nci:512 * nci + 512],
                                 in_=ps, func=AF.Exp, scale=rinv_own,
                                 accum_out=partials[:, nci:nci + 1])
        sloc = awork.tile([128, 1], f32, tag="sloc")
        nc.vector.tensor_reduce(out=sloc, in_=partials, axis=AX.X, op=ALU.add)
        nc.sync.dma_start(out=cc2_in.ap(), in_=sloc)
        nc.gpsimd.collective_compute(
            "AllReduce", ALU.add, replica_groups=G4,
            ins=[cc2_in.ap()], outs=[cc2_out.ap()])
        gsum = awork.tile([128, 1], f32, tag="gsum")
        nc.sync.dma_start(out=gsum, in_=cc2_out.ap())
        corr = awork.tile([128, 1], f32, tag="corr")
        nc.vector.reciprocal(out=corr, in_=gsum)

        # z = exp(E*corr); zsum over w; cia = xblk + z/zsum
        z_sb = big.tile([128, 8192], bf16, tag="big8k")
        nc.scalar.activation(out=z_sb, in_=E_sb, func=AF.Exp, scale=corr)
        zs = awork.tile([128, 64], f32, tag="zs")
        nc.vector.tensor_reduce(out=zs, in_=z_sb.rearrange("p (h w) -> p h w", w=128),
                                axis=AX.X, op=ALU.add)
        zr = awork.tile([128, 64], f32, tag="zr")
        nc.vector.reciprocal(out=zr, in_=zs)
        zr_b = _bcast_last(zr, 128)
        tmp = big.tile([128, 8192], bf16, tag="big8k")
        nc.vector.tensor_tensor(out=tmp.rearrange("p (h w) -> p h w", w=128),
                                in0=z_sb.rearrange("p (h w) -> p h w", w=128),
                                in1=zr_b, op=ALU.mult)
        cia_sb = big.tile([128, 8192], bf16, tag="big8k")
        nc.vector.tensor_tensor(out=cia_sb, in0=tmp, in1=xblk_sb, op=ALU.add)
        nc.sync.dma_start(out=cc3_in.ap(), in_=cia_sb)
        nc.gpsimd.collective_compute(
            "AllGather", ALU.bypass, replica_groups=G4,
            ins=[cc3_in.ap()], outs=[cc3_out.ap()])

        # ================= conv helper =================
        convin_ctx = tc.tile_pool(name="convin", bufs=2)
        convin = convin_ctx.__enter__()

        def conv(load_fn, out_tag, act_func):
            """3x3 stride-2 conv + BN fold, 64 out-ch; returns SBUF (64, 64, 64)."""
            xpads = []
            for k in range(2):
                xp = convin.tile([128, 130, 130], bf16, tag="convin")
                # zero borders
                nc.vector.memset(xp[:, 0, :], 0.0)
                nc.vector.memset(xp[:, 129, :], 0.0)
                nc.vector.memset(xp[:, 1:129, 0:1], 0.0)
                nc.vector.memset(xp[:, 1:129, 129:130], 0.0)
                load_fn(k, xp)
                xpads.append(xp)
            yout = convout.tile([64, 64, 64], bf16, tag="convout", name=out_tag)
            for ohpass in range(2):
                pst = [psb() for _ in range(4)]
                for t in range(9):
                    di, dj = t // 3, t % 3
                    for k in range(2):
                        for c4 in range(4):
                            ohc = 4 * ohpass + c4
                            rhs = xpads[k][:, di + 16 * ohc: di + 16 * ohc + 16:2,
                                           dj: dj + 128:2]
                            nc.tensor.matmul(
                                pst[c4][0:64, :].rearrange("p (a b) -> p a b", a=8),
                                lhsT=wt_sb[:, k, t, :], rhs=rhs,
                                start=(t == 0 and k == 0),
                                stop=(t == 8 and k == 1))
                for c4 in range(4):
                    ohc = 4 * ohpass + c4
                    nc.scalar.activation(
                        out=yout[:, 8 * ohc: 8 * ohc + 8, :],
                        in_=pst[c4][0:64, :].rearrange("p (a b) -> p a b", a=8),
                        func=act_func, bias=shift_sb, scale=scale_sb)
            return yout

        # ---- conv(x): loader from xb ----
        def load_x(k, xp):
            nc.sync.dma_start(out=xp[:, 1:129, 1:129],
                              in_=xb.ap()[128 * k:128 * k + 128, :, :])

        y_sb = conv(load_x, "y", mybir.ActivationFunctionType.Identity)

        # ---- conv(cia): loader from cc3_out ----
        def load_cia(k, xp):
            dst = xp.rearrange("(a b) i j -> a b i j", b=2)
            for hh in range(2):
                for par in range(2):
                    s_ap = _rap(cc3_out,
                                (par * 2 + hh) * 1048576 + (64 * k) * 8192,
                                [[8192, 64], [128, 64], [1, 128]])
                    nc.sync.dma_start(
                        out=dst[:, par, 1 + 64 * hh: 65 + 64 * hh, 1:129],
                        in_=s_ap)

        yc_sb = conv(load_cia, "yc", mybir.ActivationFunctionType.Identity)
        convin_ctx.__exit__(None, None, None)
        twork = ctx.enter_context(tc.tile_pool(name="twork", bufs=1))
        attn = ctx.enter_context(tc.tile_pool(name="attn", bufs=2))

        # ================= transposes to (ow, oh, co) =================
        def transpose_64x64(src, dst_tag):
            dst = twork.tile([64, 64, 64], bf16, tag=dst_tag)
            for g in range(8):
                ps = psbt()
                for i in range(8):
                    oh = 8 * g + i
                    nc.tensor.transpose(ps[0:64, 64 * i:64 * i + 64],
                                        in_=src[:, oh, :], identity=id64)
                nc.vector.tensor_copy(
                    out=dst[:, 8 * g:8 * g + 8, :],
                    in_=ps[0:64, :].rearrange("p (a b) -> p a b", a=8))
            return dst

        yT = transpose_64x64(y_sb, "yT")
        ycT = transpose_64x64(yc_sb, "ycT")

        brT = twork.tile([64, 64, 64], bf16, tag="brT")
        nc.scalar.activation(out=brT, in_=yT, func=AF.Relu)
        x12T = twork.tile([64, 64, 64], bf16, tag="x12T")
        nc.scalar.activation(out=x12T, in_=ycT, func=AF.Relu)
        tmpT = twork.tile([64, 64, 64], bf16, tag="tmpT")
        nc.scalar.activation(out=tmpT, in_=yT, func=AF.Lrelu, alpha=0.2)
        x43T = twork.tile([64, 64, 64], bf16, tag="x43T")
        nc.scalar.activation(out=x43T, in_=tmpT, func=AF.Sigmoid)

        # ================= attention + upsample, 8 pairs/group =========
        for g in range(8):
            def pslc(c):
                return slice(64 * c, 64 * c + 64)

            # --- att1 logits ---
            ps1 = psb()
            for i in range(8):
                c = 8 * g + i
                nc.tensor.matmul(ps1[0:64, pslc(i)], lhsT=x12T[:, :, c],
                                 rhs=brT[:, :, c], start=True, stop=True)
            # shifted softmax (batched)
            v1 = ps1[0:64, :].rearrange("p (a b) -> p a b", a=8)
            nm1 = attn.tile([64, 8], f32, tag="nm")
            nc.vector.tensor_reduce(out=nm1, in_=v1, axis=AX.X, op=ALU.max,
                                    negate=True)
            lg1 = attn.tile([64, 8, 64], bf16, tag="lg")
            nc.vector.tensor_tensor(out=lg1, in0=v1, in1=_bcast_last(nm1, 64),
                                    op=ALU.add)
            e1 = attn.tile([64, 8, 64], bf16, tag="e1")
            nc.scalar.activation(out=e1, in_=lg1, func=AF.Exp)
            s1 = attn.tile([64, 8], f32, tag="s1")
            nc.vector.tensor_reduce(out=s1, in_=e1, axis=AX.X, op=ALU.add)
            r1 = attn.tile([64, 8], f32, tag="r1")
            nc.vector.reciprocal(out=r1, in_=s1)
            att1sm = attn.tile([64, 8, 64], bf16, tag="att1sm")
            nc.vector.tensor_tensor(out=att1sm, in0=e1, in1=_bcast_last(r1, 64),
                                    op=ALU.mult)

            # --- att2 logits ---
            ps2 = psb()
            for i in range(8):
                c = 8 * g + i
                nc.tensor.matmul(ps2[0:64, pslc(i)], lhsT=brT[:, :, c],
                                 rhs=x43T[:, :, c], start=True, stop=True)
            v2 = ps2[0:64, :].rearrange("p (a b) -> p a b", a=8)
            nm2 = attn.tile([64, 8], f32, tag="nm")
            nc.vector.tensor_reduce(out=nm2, in_=v2, axis=AX.X, op=ALU.max,
                                    negate=True)
            lg2 = attn.tile([64, 8, 64], bf16, tag="lg")
            nc.vector.tensor_tensor(out=lg2, in0=v2, in1=_bcast_last(nm2, 64),
                                    op=ALU.add)
            e2 = attn.tile([64, 8, 64], bf16, tag="e2")
            nc.scalar.activation(out=e2, in_=lg2, func=AF.Exp)
            s2 = attn.tile([64, 8], f32, tag="s1")
            nc.vector.tensor_reduce(out=s2, in_=e2, axis=AX.X, op=ALU.add)
            r2 = attn.tile([64, 8], f32, tag="r1")
            nc.vector.reciprocal(out=r2, in_=s2)
            att2sm = attn.tile([64, 8, 64], bf16, tag="att2sm")
            nc.vector.tensor_tensor(out=att2sm, in0=e2, in1=_bcast_last(r2, 64),
                                    op=ALU.mult)

            # --- att2^T per pair ---
            psT2 = psbt()
            for i in range(8):
                nc.tensor.transpose(psT2[0:64, pslc(i)], in_=att2sm[:, i, :],
                                    identity=id64)
            a2T = attn.tile([64, 8, 64], bf16, tag="a2T")
            nc.vector.tensor_copy(out=a2T,
                                  in_=psT2[0:64, :].rearrange("p (a b) -> p a b", a=8))

            # --- x3_3 logits (shift-free) ---
            ps3 = psb()
            for i in range(8):
                c = 8 * g + i
                nc.tensor.matmul(ps3[0:64, pslc(i)], lhsT=x12T[:, :, c],
                                 rhs=a2T[:, i, :], start=True, stop=True)
            e3 = attn.tile([64, 8, 64], bf16, tag="e1")
            nc.scalar.activation(out=e3,
                                 in_=ps3[0:64, :].rearrange("p (a b) -> p a b", a=8),
                                 func=AF.Exp)
            s3 = attn.tile([64, 8], f32, tag="s1")
            nc.vector.tensor_reduce(out=s3, in_=e3, axis=AX.X, op=ALU.add)
            r3 = attn.tile([64, 8], f32, tag="r1")
            nc.vector.reciprocal(out=r3, in_=s3)
            x33sm = attn.tile([64, 8, 64], f32, tag="x33sm")
            nc.vector.tensor_tensor(out=x33sm, in0=e3, in1=_bcast_last(r3, 64),
                                    op=ALU.mult)

            # --- branch natural layout (oh, ow) per pair ---
            psB2 = psbt()
            for i in range(8):
                c = 8 * g + i
                nc.tensor.transpose(psB2[0:64, pslc(i)], in_=brT[:, :, c],
                                    identity=id64)

            # --- S = relu(x33sm + att1sm + br_nat) ---
            t2 = attn.tile([64, 8, 64], f32, tag="t2", bufs=1)
            nc.vector.tensor_tensor(out=t2, in0=x33sm, in1=att1sm, op=ALU.add)
            t3 = attn.tile([64, 8, 64], f32, tag="t3", bufs=1)
            nc.vector.tensor_tensor(
                out=t3, in0=t2,
                in1=psB2[0:64, :].rearrange("p (a b) -> p a b", a=8), op=ALU.add)
            S_sb = attn.tile([64, 8, 64], bf16, tag="S")
            nc.vector.tensor_scalar_max(S_sb, t3, 0.0)

            # --- upsample: W1 = R @ S ; U = R @ W1^T (== W1 @ R^T) ---
            psW = psb()
            for i in range(8):
                nc.tensor.matmul(psW[:, pslc(i)], lhsT=rt_sb, rhs=S_sb[:, i, :],
                                 start=True, stop=True)
            W1 = attn.tile([128, 8, 64], bf16, tag="W1")
            nc.vector.tensor_copy(out=W1,
                                  in_=psW.rearrange("p (a b) -> p a b", a=8))
            psT = [psbt(), psbt()]
            for i in range(8):
                nc.tensor.transpose(psT[i // 4][0:64, 128 * (i % 4):128 * (i % 4) + 128],
                                    in_=W1[:, i, :], identity=id128)
            W1T = attn.tile([64, 8, 128], bf16, tag="W1T")
            for h in range(2):
                nc.vector.tensor_copy(
                    out=W1T[:, 4 * h:4 * h + 4, :],
                    in_=psT[h][0:64, :].rearrange("p (a b) -> p a b", a=4))
            psU = [psb(), psb()]
            for i in range(8):
                nc.tensor.matmul(psU[i // 4][:, 128 * (i % 4):128 * (i % 4) + 128],
                                 lhsT=W1T[:, i, :], rhs=rt_sb,
                                 start=True, stop=True)
            for h in range(2):
                u_sb = attn.tile([128, 4, 128], bf16, tag="u")
                nc.vector.tensor_copy(
                    out=u_sb, in_=psU[h].rearrange("p (a b) -> p a b", a=4))
                c0 = 8 * g + 4 * h
                nc.sync.dma_start(
                    out=out_sh.ap()[c0:c0 + 4, :, :].rearrange("c h w -> h c w"),
                    in_=u_sb)

    nc.finalize()
    return nc


def host_prep(inputs):
    """Build per-core in_maps (host does only slicing/transpose/cast)."""
    bfd = ml_dtypes.bfloat16
    x = np.asarray(inputs["x"], np.float32)
    conv_w = np.asarray(inputs["conv_w"], np.float32)
    conv_b = np.asarray(inputs["conv_b"], np.float32)
    g = np.asarray(inputs["bn_gamma"], np.float32)
    be = np.asarray(inputs["bn_beta"], np.float32)
    mu = np.asarray(inputs["bn_mean"], np.float32)
    var = np.asarray(inputs["bn_var"], np.float32)

    scale = (g / np.sqrt(var + EPS)).astype(np.float32)
    shift = ((conv_b - mu) * scale + be).astype(np.float32)
    Xc = x.reshape(C, -1)
    Xr = x.reshape(-1, C)
    RT = np.ascontiguousarray(_resize_mat(H, HO).T).astype(bfd)
    wtr = conv_w.transpose(1, 0, 2, 3)  # (ci, co, 3, 3)

    in_maps = []
    for p in range(N_CORES):
        b, q = p // 4, p % 4
        parity, hh = q // 2, q % 2
        xcT_h = np.ascontiguousarray(Xc[:, 4096 * p:4096 * (p + 1)].T).astype(bfd)
        xr_h = np.ascontiguousarray(Xr[4096 * p:4096 * (p + 1), :]).astype(bfd)
        xcm2_h = np.ascontiguousarray(
            Xc[:, 8192 * q:8192 * (q + 1)].reshape(2, 128, 8192)
            .transpose(1, 0, 2)).astype(bfd)
        xblk_h = np.ascontiguousarray(xcm2_h[:, b, :])
        sel_h = np.zeros((128, 2, 128), np.float32)
        sel_h[np.arange(128), b, np.arange(128)] = 1.0
        wt_h = np.ascontiguousarray(
            wtr[:, 64 * q:64 * q + 64].reshape(2, 128, 64, 9)
            .transpose(1, 0, 3, 2)).astype(bfd)
        in_maps.append({
            "xb": x[b].astype(bfd),
            "xcT": xcT_h,
            "xr": xr_h,
            "xcm2": xcm2_h,
            "xblk": xblk_h,
            "sel": sel_h.astype(bfd),
            "wt": wt_h,
            "scale": scale[64 * q:64 * q + 64].reshape(64, 1),
            "shift": shift[64 * q:64 * q + 64].reshape(64, 1),
            "rt": RT,
        })
    return in_maps


_NC = None


def _get_nc():
    global _NC
    if _NC is None:
        _NC = build_nc()
    return _NC


def run_device(inputs, trace=False):
    from concourse.bass_utils import run_bass_kernel_spmd
    nc = _get_nc()
    in_maps = host_prep(inputs)
    res = run_bass_kernel_spmd(nc, in_maps, list(range(N_CORES)), trace=trace)
    out = np.zeros((B, C, H, W), np.float32)
    for p in range(N_CORES):
        b, q = p // 4, p % 4
        out[b, 64 * q:64 * q + 64] = res.results[p]["out_sh"].astype(np.float32)
    return out, res


def kernel(**inputs):
    out, _ = run_device(inputs, trace=False)
    return out



# revision 1
# speedup vs baseline: 7487.5384x; 7487.5384x over previous
"""Self-contained Bass/Tile SPMD kernel for nn_AIA_1_56049323213170 (8 NeuronCores).

Pipeline (B=2, C=256, H=W=128), all heavy math on-device in bf16/f32-psum:
  M1 = Xc @ Xr (CxC, contraction sharded 8-way + AllReduce)
  a  = rowsoftmax(M1)  (redundant per core, unnormalized + row-recip trick)
  s  = a @ Xc          (sharded: rows by batch, cols by quarter -> (128, 8192))
  rowsoftmax(s) needs only a global row-sum (|s|<=5.5 -> shift-free exp):
       AllReduce of per-core row sums within each batch group of 4 cores
  cia = x + softmax_W(rowsoftmax(s))   (W-softmax local, shift-free)
  AllGather cia within batch group -> full cia[b] per core
  y   = BN(conv3x3s2(x)+b), yc = BN(conv3x3s2(cia)+b)   (64 out-ch per core)
  branch = relu(y); x4_3 = sigmoid(lrelu(y,.2)); x1_2 = relu(yc)
  att1 = rowsoftmax(x1_2 @ branch^T); att2 = rowsoftmax(branch @ x4_3^T)
  x3_3 = rowsoftmax(x1_2 @ att2^T)  (shift-free)
  out  = bilinear_up2(relu(x3_3 + att1 + branch)) via R @ S @ R^T

Core p: b = p//4 (batch), q = p%4 (quarter; parity=q//2, h-half=q%2).
Each core returns out[b, 64q:64q+64] as bf16; host assembles f32.
"""
import numpy as np
import ml_dtypes

N_CORES = 8
B, C, H, W = 2, 256, 128, 128
HO = WO = 64
EPS = 1e-5

F32 = None  # set after imports
BF16 = None


def _resize_mat(n_out, n_in):
    R = np.zeros((n_out, n_in), np.float32)
    scale = n_in / n_out
    for i in range(n_out):
        src = (i + 0.5) * scale - 0.5
        i0 = int(np.floor(src))
        frac = src - i0
        lo = min(max(i0, 0), n_in - 1)
        hi = min(max(i0 + 1, 0), n_in - 1)
        R[i, lo] += 1.0 - frac
        R[i, hi] += frac
    return R


def _ap_of(t):
    """Normalize a tensor handle / tile / AP to a bass.AP."""
    import concourse.bass as bass
    if isinstance(t, bass.AP):
        return t
    return t.ap()


def _rap(t, offset, dims):
    """Raw AP over tensor handle t with [step,count] dims (elements)."""
    import concourse.bass as bass
    base = _ap_of(t)
    return bass.AP(tensor=base.tensor, offset=base.offset + offset,
                   ap=[list(d) for d in dims])


def _bcast_last(t, n):
    """AP of 2-D tile t broadcast with a 0-step innermost dim of size n."""
    import concourse.bass as bass
    base = _ap_of(t)
    return bass.AP(tensor=base.tensor, offset=base.offset,
                   ap=[list(base.ap[0]), list(base.ap[1]), [0, n]])


def build_nc():
    from contextlib import ExitStack
    import concourse.bass as bass
    import concourse.mybir as mybir
    import concourse.tile as tile
    from concourse import bacc
    from concourse.masks import make_identity

    f32 = mybir.dt.float32
    bf16 = mybir.dt.bfloat16
    AF = mybir.ActivationFunctionType
    AX = mybir.AxisListType
    ALU = mybir.AluOpType

    nc = bacc.Bacc("TRN2", target_bir_lowering=False, debug=False,
                   num_devices=N_CORES)

    # ---- I/O ----
    xb = nc.dram_tensor("xb", [C, H, W], bf16, kind="ExternalInput")
    xcT = nc.dram_tensor("xcT", [4096, 256], bf16, kind="ExternalInput")
    xr = nc.dram_tensor("xr", [4096, 256], bf16, kind="ExternalInput")
    xcm2 = nc.dram_tensor("xcm2", [128, 2, 8192], bf16, kind="ExternalInput")
    xblk = nc.dram_tensor("xblk", [128, 8192], bf16, kind="ExternalInput")
    sel = nc.dram_tensor("sel", [128, 2, 128], bf16, kind="ExternalInput")
    wt = nc.dram_tensor("wt", [128, 2, 9, 64], bf16, kind="ExternalInput")
    scale_d = nc.dram_tensor("scale", [64, 1], f32, kind="ExternalInput")
    shift_d = nc.dram_tensor("shift", [64, 1], f32, kind="ExternalInput")
    rt_d = nc.dram_tensor("rt", [64, 128], bf16, kind="ExternalInput")
    out_sh = nc.dram_tensor("out_sh", [64, H, W], bf16, kind="ExternalOutput")

    # ---- collective scratch ----
    cc1_in = nc.dram_tensor("cc1_in", [256, 256], f32)
    cc1_out = nc.dram_tensor("cc1_out", [256, 256], f32, addr_space="Shared")
    cc2_in = nc.dram_tensor("cc2_in", [128, 1], f32)
    cc2_out = nc.dram_tensor("cc2_out", [128, 1], f32)
    cc3_in = nc.dram_tensor("cc3_in", [128, 8192], bf16)
    cc3_out = nc.dram_tensor("cc3_out", [4, 128, 8192], bf16)
    G8 = [list(range(8))]
    G4 = [[0, 1, 2, 3], [4, 5, 6, 7]]

    with tile.TileContext(nc) as tc, ExitStack() as ctx:
        consts = ctx.enter_context(tc.tile_pool(name="consts", bufs=1))
        awork = ctx.enter_context(tc.tile_pool(name="awork", bufs=1))
        big = ctx.enter_context(tc.tile_pool(name="big", bufs=3))
        convout = ctx.enter_context(tc.tile_pool(name="convout", bufs=2))
        psA = ctx.enter_context(tc.tile_pool(name="psA", bufs=4, space="PSUM"))
        psB = ctx.enter_context(tc.tile_pool(name="psB", bufs=4, space="PSUM"))

        def psa():
            return psA.tile([128, 512], f32, tag="psA", name="psA_t")

        def psb():
            return psB.tile([128, 512], f32, tag="psB", name="psB_t")

        def psbt():
            return psB.tile([128, 512], bf16, tag="psB", name="psBt_t")

        # ================= constants =================
        id64 = consts.tile([64, 64], bf16, tag="id64")
        make_identity(nc, id64)
        id128 = consts.tile([128, 128], bf16, tag="id128")
        make_identity(nc, id128)
        rt_sb = consts.tile([64, 128], bf16, tag="rt")
        nc.sync.dma_start(out=rt_sb, in_=rt_d.ap())
        sel_sb = consts.tile([128, 2, 128], bf16, tag="sel")
        nc.sync.dma_start(out=sel_sb, in_=sel.ap())
        wt_sb = consts.tile([128, 2, 9, 64], bf16, tag="wt")
        nc.sync.dma_start(out=wt_sb, in_=wt.ap())
        scale_sb = consts.tile([64, 1], f32, tag="scale")
        nc.sync.dma_start(out=scale_sb, in_=scale_d.ap())
        shift_sb = consts.tile([64, 1], f32, tag="shift")
        nc.sync.dma_start(out=shift_sb, in_=shift_d.ap())
        xcm2_sb = consts.tile([128, 2, 8192], bf16, tag="xcm2")
        nc.sync.dma_start(out=xcm2_sb, in_=xcm2.ap())
        xblk_sb = consts.tile([128, 8192], bf16, tag="xblk")
        nc.sync.dma_start(out=xblk_sb, in_=xblk.ap())

        # ================= Phase A: M1 + softmax(a) =================
        xcT_sb = big.tile([128, 32, 256], bf16, tag="big8k", name="xcT_sb")
        nc.sync.dma_start(out=xcT_sb,
                          in_=xcT.ap().rearrange("(t p) c -> p t c", p=128))
        xr_sb = big.tile([128, 32, 256], bf16, tag="big8k", name="xr_sb")
        nc.sync.dma_start(out=xr_sb,
                          in_=xr.ap().rearrange("(t p) c -> p t c", p=128))

        m1ps = [psa(), psa()]
        for mc in range(2):
            for t in range(32):
                nc.tensor.matmul(
                    m1ps[mc][:, 0:256],
                    lhsT=xcT_sb[:, t, 128 * mc:128 * mc + 128],
                    rhs=xr_sb[:, t, :],
                    start=(t == 0), stop=(t == 31))
        for mc in range(2):
            m1e = awork.tile([128, 256], f32, tag="m1e", bufs=2)
            nc.vector.tensor_copy(out=m1e, in_=m1ps[mc][:, 0:256])
            nc.sync.dma_start(out=cc1_in.ap()[128 * mc:128 * mc + 128, :],
                              in_=m1e)
        nc.gpsimd.collective_compute(
            "AllReduce", ALU.add, replica_groups=G8,
            ins=[cc1_in.ap()], outs=[cc1_out.ap()])

        a_bf = []
        rinv = []
        for ch in range(2):
            a_raw = awork.tile([128, 256], f32, tag="a_raw")
            nc.sync.dma_start(out=a_raw,
                              in_=cc1_out.ap()[128 * ch:128 * ch + 128, :])
            negmax = awork.tile([128, 1], f32, tag="negmax")
            nc.vector.tensor_reduce(out=negmax, in_=a_raw, axis=AX.X,
                                    op=ALU.max, negate=True)
            a_e = awork.tile([128, 256], bf16, tag="a_e", bufs=2)
            asum = awork.tile([128, 1], f32, tag="asum")
            nc.scalar.activation(out=a_e, in_=a_raw, func=AF.Exp,
                                 bias=negmax, scale=1.0, accum_out=asum)
            ri = awork.tile([128, 1], f32, tag="ri", bufs=2)
            nc.vector.reciprocal(out=ri, in_=asum)
            a_bf.append(a_e)
            rinv.append(ri)

        # aT_own[c2,m] = sum_c1 a[c1,c2]*sel[c1,m];  rinv_own = sel^T @ rinv
        aTo = []
        for c2c in range(2):
            ps = psa()
            for c1c in range(2):
                nc.tensor.matmul(ps[:, 0:128],
                                 lhsT=a_bf[c1c][:, 128 * c2c:128 * c2c + 128],
                                 rhs=sel_sb[:, c1c, :],
                                 start=(c1c == 0), stop=(c1c == 1))
            t_ = awork.tile([128, 128], bf16, tag="aTo", bufs=2)
            nc.vector.tensor_copy(out=t_, in_=ps[:, 0:128])
            aTo.append(t_)
        sel_f = awork.tile([128, 2, 128], f32, tag="sel_f")
        nc.vector.tensor_copy(out=sel_f, in_=sel_sb)
        ps = psa()
        for c1c in range(2):
            nc.tensor.matmul(ps[:, 0:1], lhsT=sel_f[:, c1c, :],
                             rhs=rinv[c1c], start=(c1c == 0), stop=(c1c == 1))
        rinv_own = awork.tile([128, 1], f32, tag="rinv_own")
        nc.vector.tensor_copy(out=rinv_own, in_=ps[:, 0:1])

        # ================= Phase B: M2 -> E = exp(s) =================
        E_sb = big.tile([128, 8192], bf16, tag="big8k")
        partials = awork.tile([128, 16], f32, tag="partials")
        for nci in range(16):
            ps = psa()
            for bp in range(2):
                nc.tensor.matmul(ps,
                                 lhsT=aTo[bp],
                                 rhs=xcm2_sb[:, bp, 512 * nci:512 * nci + 512],
                                 start=(bp == 0), stop=(bp == 1))
            nc.scalar.activation(out=E_sb[:, 512 * nci:512 * nci + 512],
                                 in_=ps, func=AF.Exp, scale=rinv_own,
                                 accum_out=partials[:, nci:nci + 1])
        sloc = awork.tile([128, 1], f32, tag="sloc")
        nc.vector.tensor_reduce(out=sloc, in_=partials, axis=AX.X, op=ALU.add)
        nc.sync.dma_start(out=cc2_in.ap(), in_=sloc)
        nc.gpsimd.collective_compute(
            "AllReduce", ALU.add, replica_groups=G4,
            ins=[cc2_in.ap()], outs=[cc2_out.ap()])
        gsum = awork.tile([128, 1], f32, tag="gsum")
        nc.sync.dma_start(out=gsum, in_=cc2_out.ap())
        corr = awork.tile([128, 1], f32, tag="corr")
        nc.vector.reciprocal(out=corr, in_=gsum)

        # z = exp(E*corr); zsum over w; cia = xblk + z/zsum
        z_sb = big.tile([128, 8192], bf16, tag="big8k")
        nc.scalar.activation(out=z_sb, in_=E_sb, func=AF.Exp, scale=corr)
        zs = awork.tile([128, 64], f32, tag="zs")
        nc.vector.tensor_reduce(out=zs, in_=z_sb.rearrange("p (h w) -> p h w", w=128),
                                axis=AX.X, op=ALU.add)
        zr = awork.tile([128, 64], f32, tag="zr")
        nc.vector.reciprocal(out=zr, in_=zs)
        zr_b = _bcast_last(zr, 128)
        tmp = big.tile([128, 8192], bf16, tag="big8k")
        nc.vector.tensor_tensor(out=tmp.rearrange("p (h w) -> p h w", w=128),
                                in0=z_sb.rearrange("p (h w) -> p h w", w=128),
                                in1=zr_b, op=ALU.mult)
        cia_sb = big.tile([128, 8192], bf16, tag="big8k")
        nc.vector.tensor_tensor(out=cia_sb, in0=tmp, in1=xblk_sb, op=ALU.add)
        nc.sync.dma_start(out=cc3_in.ap(), in_=cia_sb)
        nc.gpsimd.collective_compute(
            "AllGather", ALU.bypass, replica_groups=G4,
            ins=[cc3_in.ap()], outs=[cc3_out.ap()])

        # ================= conv helper =================
        convin_ctx = tc.tile_pool(name="convin", bufs=2)
        convin = convin_ctx.__enter__()

        def conv(load_fn, out_tag, act_func):
            """3x3 stride-2 conv + BN fold, 64 out-ch; returns SBUF (64, 64, 64)."""
            xpads = []
            for k in range(2):
                xp = convin.tile([128, 130, 130], bf16, tag="convin")
                # zero borders
                nc.vector.memset(xp[:, 0, :], 0.0)
                nc.vector.memset(xp[:, 129, :], 0.0)
                nc.vector.memset(xp[:, 1:129, 0:1], 0.0)
                nc.vector.memset(xp[:, 1:129, 129:130], 0.0)
                load_fn(k, xp)
                xpads.append(xp)
            yout = convout.tile([64, 64, 64], bf16, tag="convout", name=out_tag)
            for ohpass in range(2):
                pst = [psb() for _ in range(4)]
                for t in range(9):
                    di, dj = t // 3, t % 3
                    for k in range(2):
                        for c4 in range(4):
                            ohc = 4 * ohpass + c4
                            rhs = xpads[k][:, di + 16 * ohc: di + 16 * ohc + 16:2,
                                           dj: dj + 128:2]
                            nc.tensor.matmul(
                                pst[c4][0:64, :].rearrange("p (a b) -> p a b", a=8),
                                lhsT=wt_sb[:, k, t, :], rhs=rhs,
                                start=(t == 0 and k == 0),
                                stop=(t == 8 and k == 1))
                for c4 in range(4):
                    ohc = 4 * ohpass + c4
                    nc.scalar.activation(
                        out=yout[:, 8 * ohc: 8 * ohc + 8, :],
                        in_=pst[c4][0:64, :].rearrange("p (a b) -> p a b", a=8),
                        func=act_func, bias=shift_sb, scale=scale_sb)
            return yout

        # ---- conv(x): loader from xb ----
        def load_x(k, xp):
            nc.sync.dma_start(out=xp[:, 1:129, 1:129],
                              in_=xb.ap()[128 * k:128 * k + 128, :, :])

        y_sb = conv(load_x, "y", mybir.ActivationFunctionType.Identity)

        # ---- conv(cia): loader from cc3_out ----
        def load_cia(k, xp):
            dst = xp.rearrange("(a b) i j -> a b i j", b=2)
            for hh in range(2):
                for par in range(2):
                    s_ap = _rap(cc3_out,
                                (par * 2 + hh) * 1048576 + (64 * k) * 8192,
                                [[8192, 64], [128, 64], [1, 128]])
                    nc.sync.dma_start(
                        out=dst[:, par, 1 + 64 * hh: 65 + 64 * hh, 1:129],
                        in_=s_ap)

        yc_sb = conv(load_cia, "yc", mybir.ActivationFunctionType.Identity)
        convin_ctx.__exit__(None, None, None)
        twork = ctx.enter_context(tc.tile_pool(name="twork", bufs=1))
        attn = ctx.enter_context(tc.tile_pool(name="attn", bufs=2))

        # ================= transposes to (ow, oh, co) =================
        def transpose_64x64(src, dst_tag):
            dst = twork.tile([64, 64, 64], bf16, tag=dst_tag)
            for g in range(8):
                ps = psbt()
                for i in range(8):
                    oh = 8 * g + i
                    nc.tensor.transpose(ps[0:64, 64 * i:64 * i + 64],
                                        in_=src[:, oh, :], identity=id64)
                nc.vector.tensor_copy(
                    out=dst[:, 8 * g:8 * g + 8, :],
                    in_=ps[0:64, :].rearrange("p (a b) -> p a b", a=8))
            return dst

        yT = transpose_64x64(y_sb, "yT")
        ycT = transpose_64x64(yc_sb, "ycT")

        brT = twork.tile([64, 64, 64], bf16, tag="brT")
        nc.scalar.activation(out=brT, in_=yT, func=AF.Relu)
        x12T = twork.tile([64, 64, 64], bf16, tag="x12T")
        nc.scalar.activation(out=x12T, in_=ycT, func=AF.Relu)
        tmpT = twork.tile([64, 64, 64], bf16, tag="tmpT")
        nc.scalar.activation(out=tmpT, in_=yT, func=AF.Lrelu, alpha=0.2)
        x43T = twork.tile([64, 64, 64], bf16, tag="x43T")
        nc.scalar.activation(out=x43T, in_=tmpT, func=AF.Sigmoid)

        # ================= attention + upsample, 8 pairs/group =========
        for g in range(8):
            def pslc(c):
                return slice(64 * c, 64 * c + 64)

            # --- att1 logits ---
            ps1 = psb()
            for i in range(8):
                c = 8 * g + i
                nc.tensor.matmul(ps1[0:64, pslc(i)], lhsT=x12T[:, :, c],
                                 rhs=brT[:, :, c], start=True, stop=True)
            # shifted softmax (batched)
            v1 = ps1[0:64, :].rearrange("p (a b) -> p a b", a=8)
            nm1 = attn.tile([64, 8], f32, tag="nm")
            nc.vector.tensor_reduce(out=nm1, in_=v1, axis=AX.X, op=ALU.max,
                                    negate=True)
            lg1 = attn.tile([64, 8, 64], bf16, tag="lg")
            nc.vector.tensor_tensor(out=lg1, in0=v1, in1=_bcast_last(nm1, 64),
                                    op=ALU.add)
            e1 = attn.tile([64, 8, 64], bf16, tag="e1")
            nc.scalar.activation(out=e1, in_=lg1, func=AF.Exp)
            s1 = attn.tile([64, 8], f32, tag="s1")
            nc.vector.tensor_reduce(out=s1, in_=e1, axis=AX.X, op=ALU.add)
            r1 = attn.tile([64, 8], f32, tag="r1")
            nc.vector.reciprocal(out=r1, in_=s1)
            att1sm = attn.tile([64, 8, 64], bf16, tag="att1sm")
            nc.vector.tensor_tensor(out=att1sm, in0=e1, in1=_bcast_last(r1, 64),
                                    op=ALU.mult)

            # --- att2 logits ---
            ps2 = psb()
            for i in range(8):
                c = 8 * g + i
                nc.tensor.matmul(ps2[0:64, pslc(i)], lhsT=brT[:, :, c],
                                 rhs=x43T[:, :, c], start=True, stop=True)
            v2 = ps2[0:64, :].rearrange("p (a b) -> p a b", a=8)
            nm2 = attn.tile([64, 8], f32, tag="nm")
            nc.vector.tensor_reduce(out=nm2, in_=v2, axis=AX.X, op=ALU.max,
                                    negate=True)
            lg2 = attn.tile([64, 8, 64], bf16, tag="lg")
            nc.vector.tensor_tensor(out=lg2, in0=v2, in1=_bcast_last(nm2, 64),
                                    op=ALU.add)
            e2 = attn.tile([64, 8, 64], bf16, tag="e2")
            nc.scalar.activation(out=e2, in_=lg2, func=AF.Exp)
            s2 = attn.tile([64, 8], f32, tag="s1")
            nc.vector.tensor_reduce(out=s2, in_=e2, axis=AX.X, op=ALU.add)
            r2 = attn.tile([64, 8], f32, tag="r1")
            nc.vector.reciprocal(out=r2, in_=s2)
            att2sm = attn.tile([64, 8, 64], bf16, tag="att2sm")
            nc.vector.tensor_tensor(out=att2sm, in0=e2, in1=_bcast_last(r2, 64),
                                    op=ALU.mult)

            # --- att2^T per pair ---
            psT2 = psbt()
            for i in range(8):
                nc.tensor.transpose(psT2[0:64, pslc(i)], in_=att2sm[:, i, :],
                                    identity=id64)
            a2T = attn.tile([64, 8, 64], bf16, tag="a2T")
            nc.vector.tensor_copy(out=a2T,
                                  in_=psT2[0:64, :].rearrange("p (a b) -> p a b", a=8))

            # --- x3_3 logits (shift-free) ---
            ps3 = psb()
            for i in range(8):
                c = 8 * g + i
                nc.tensor.matmul(ps3[0:64, pslc(i)], lhsT=x12T[:, :, c],
                                 rhs=a2T[:, i, :], start=True, stop=True)
            e3 = attn.tile([64, 8, 64], bf16, tag="e1")
            nc.scalar.activation(out=e3,
                                 in_=ps3[0:64, :].rearrange("p (a b) -> p a b", a=8),
                                 func=AF.Exp)
            s3 = attn.tile([64, 8], f32, tag="s1")
            nc.vector.tensor_reduce(out=s3, in_=e3, axis=AX.X, op=ALU.add)
            r3 = attn.tile([64, 8], f32, tag="r1")
            nc.vector.reciprocal(out=r3, in_=s3)
            x33sm = attn.tile([64, 8, 64], f32, tag="x33sm")
            nc.vector.tensor_tensor(out=x33sm, in0=e3, in1=_bcast_last(r3, 64),
                                    op=ALU.mult)

            # --- branch natural layout (oh, ow) per pair ---
            psB2 = psbt()
            for i in range(8):
                c = 8 * g + i
                nc.tensor.transpose(psB2[0:64, pslc(i)], in_=brT[:, :, c],
                                    identity=id64)

            # --- S = relu(x33sm + att1sm + br_nat) ---
            t2 = attn.tile([64, 8, 64], f32, tag="t2", bufs=1)
            nc.vector.tensor_tensor(out=t2, in0=x33sm, in1=att1sm, op=ALU.add)
            t3 = attn.tile([64, 8, 64], f32, tag="t3", bufs=1)
            nc.vector.tensor_tensor(
                out=t3, in0=t2,
                in1=psB2[0:64, :].rearrange("p (a b) -> p a b", a=8), op=ALU.add)
            S_sb = attn.tile([64, 8, 64], bf16, tag="S")
            nc.vector.tensor_scalar_max(S_sb, t3, 0.0)

            # --- upsample: W1 = R @ S ; U = R @ W1^T (== W1 @ R^T) ---
            psW = psb()
            for i in range(8):
                nc.tensor.matmul(psW[:, pslc(i)], lhsT=rt_sb, rhs=S_sb[:, i, :],
                                 start=True, stop=True)
            W1 = attn.tile([128, 8, 64], bf16, tag="W1")
            nc.vector.tensor_copy(out=W1,
                                  in_=psW.rearrange("p (a b) -> p a b", a=8))
            psT = [psbt(), psbt()]
            for i in range(8):
                nc.tensor.transpose(psT[i // 4][0:64, 128 * (i % 4):128 * (i % 4) + 128],
                                    in_=W1[:, i, :], identity=id128)
            W1T = attn.tile([64, 8, 128], bf16, tag="W1T")
            for h in range(2):
                nc.vector.tensor_copy(
                    out=W1T[:, 4 * h:4 * h + 4, :],
                    in_=psT[h][0:64, :].rearrange("p (a b) -> p a b", a=4))
            psU = [psb(), psb()]
            for i in range(8):
                nc.tensor.matmul(psU[i // 4][:, 128 * (i % 4):128 * (i % 4) + 128],
                                 lhsT=W1T[:, i, :], rhs=rt_sb,
                                 start=True, stop=True)
            for h in range(2):
                u_sb = attn.tile([128, 4, 128], bf16, tag="u")
                nc.vector.tensor_copy(
                    out=u_sb, in_=psU[h].rearrange("p (a b) -> p a b", a=4))
                c0 = 8 * g + 4 * h
                nc.sync.dma_start(
                    out=out_sh.ap()[c0:c0 + 4, :, :].rearrange("c h w -> h c w"),
                    in_=u_sb)

    nc.finalize()
    return nc


def host_prep(inputs):
    """Build per-core in_maps (host does only slicing/transpose/cast)."""
    bfd = ml_dtypes.bfloat16
    x = np.asarray(inputs["x"], np.float32)
    conv_w = np.asarray(inputs["conv_w"], np.float32)
    conv_b = np.asarray(inputs["conv_b"], np.float32)
    g = np.asarray(inputs["bn_gamma"], np.float32)
    be = np.asarray(inputs["bn_beta"], np.float32)
    mu = np.asarray(inputs["bn_mean"], np.float32)
    var = np.asarray(inputs["bn_var"], np.float32)

    scale = (g / np.sqrt(var + EPS)).astype(np.float32)
    shift = ((conv_b - mu) * scale + be).astype(np.float32)
    Xc = x.reshape(C, -1)
    Xr = x.reshape(-1, C)
    RT = np.ascontiguousarray(_resize_mat(H, HO).T).astype(bfd)
    wtr = conv_w.transpose(1, 0, 2, 3)  # (ci, co, 3, 3)

    in_maps = []
    for p in range(N_CORES):
        b, q = p // 4, p % 4
        parity, hh = q // 2, q % 2
        xcT_h = np.ascontiguousarray(Xc[:, 4096 * p:4096 * (p + 1)].T).astype(bfd)
        xr_h = np.ascontiguousarray(Xr[4096 * p:4096 * (p + 1), :]).astype(bfd)
        xcm2_h = np.ascontiguousarray(
            Xc[:, 8192 * q:8192 * (q + 1)].reshape(2, 128, 8192)
            .transpose(1, 0, 2)).astype(bfd)
        xblk_h = np.ascontiguousarray(xcm2_h[:, b, :])
        sel_h = np.zeros((128, 2, 128), np.float32)
        sel_h[np.arange(128), b, np.arange(128)] = 1.0
        wt_h = np.ascontiguousarray(
            wtr[:, 64 * q:64 * q + 64].reshape(2, 128, 64, 9)
            .transpose(1, 0, 3, 2)).astype(bfd)
        in_maps.append({
            "xb": x[b].astype(bfd),
            "xcT": xcT_h,
            "xr": xr_h,
            "xcm2": xcm2_h,
            "xblk": xblk_h,
            "sel": sel_h.astype(bfd),
            "wt": wt_h,
            "scale": scale[64 * q:64 * q + 64].reshape(64, 1),
            "shift": shift[64 * q:64 * q + 64].reshape(64, 1),
            "rt": RT,
        })
    return in_maps


_NC = None


def _get_nc():
    global _NC
    if _NC is None:
        _NC = build_nc()
    return _NC


def run_device(inputs, trace=False):
    from concourse.bass_utils import run_bass_kernel_spmd
    nc = _get_nc()
    in_maps = host_prep(inputs)
    res = run_bass_kernel_spmd(nc, in_maps, list(range(N_CORES)), trace=trace)
    out = np.zeros((B, C, H, W), np.float32)
    for p in range(N_CORES):
        b, q = p // 4, p % 4
        out[b, 64 * q:64 * q + 64] = res.results[p]["out_sh"].astype(np.float32)
    return out, res


def kernel(**inputs):
    out, _ = run_device(inputs, trace=False)
    return out

